# revision 8
# baseline (speedup 1.0000x reference)
"""DCT-II (norm='ortho') along axis 1 of x[8, 4096, 1024] on 8 NeuronCores.

Batch-parallel: core c transforms batch c. Depth-4 factorization:
  II_4096 -(butterfly)-> II_2048 + IV_2048
  II_2048 -(butterfly)-> II_1024 + IV_1024
  IV_2048 -(rotation V2)-> IV_1024(a) + IV_1024(b~)   [DST sign-folds in V2]
  II_1024 -(butterfly)-> II_512(u3) + IV_512(v3)
  IV_1024 -(rotation V3)-> IV_512 + IV_512            [x3 in {v2, a, b~}]
Leaves use 3 distinct 512x512 fp16 matrices (CII, CIV, J@CIV with ortho
scaling folded in); outputs assemble with lane-aligned +/- combos into
stride-8 row groups of y. Row reversals / scaled reversals run on the
tensor engine as (scaled) anti-identity matmuls; butterflies and
rotations run as fp16 tensor_tensor / scalar_tensor_tensor ops split
across Vector/Scalar/GpSimd. x and y travel as fp16 (host converts).
"""

import sys

sys.path.insert(0, "/opt/trn_rl_repo")
import numpy as np

B, S, D = 8, 4096, 1024

_cache: dict = {}


# ---------------------------------------------------------------- host math
def _CII(n):
    k = np.arange(n)[:, None]
    m = np.arange(n)[None, :]
    return np.cos(np.pi * (2 * m + 1) * k / (2 * n))


def _CIV(n):
    k = np.arange(n)[:, None]
    m = np.arange(n)[None, :]
    return np.cos(np.pi * (2 * m + 1) * (2 * k + 1) / (4 * n))


def _DSTIV(n):
    k = np.arange(n)[:, None]
    m = np.arange(n)[None, :]
    return np.sin(np.pi * (2 * m + 1) * (2 * k + 1) / (4 * n))


def _solve_iv_split(n):
    m = n // 2
    C, Sm = _CIV(m), _DSTIV(m)
    Minv = np.zeros((n, n))
    Minv[:m, :m] = (2.0 / m) * C.T
    Minv[m:, m:] = (2.0 / m) * Sm.T
    P = np.zeros((n, n))
    i = np.arange(m)
    P[2 * i, i] = 1
    P[2 * i, m + i] = 1
    P[2 * i + 1, i] = 1
    P[2 * i + 1, m + i] = -1
    R = Minv @ (P.T / 2) @ _CIV(n)
    cA = R[i, i].copy()
    sA = R[i, n - 1 - i].copy()
    cB = R[m + i, i].copy()
    sB = R[m + i, n - 1 - i].copy()
    return cA, sA, cB, sB


def _blocked(c):
    # [kt][i, nt*128+j] = C[kt*128+j, nt*128+i]  (lhsT blocks per output tile)
    kt = c.shape[0] // 128
    nt = c.shape[1] // 128
    ct = c.T.astype(np.float16).reshape(nt, 128, kt, 128).transpose(2, 1, 0, 3)
    return np.ascontiguousarray(ct.reshape(kt, 128, nt * 128))


def _scaled_j_blocks(svec):
    # per 128-tile q: lhsT[k, m] = svec[q*128+m] if k == 127-m else 0
    T = len(svec) // 128
    out = np.zeros((T, 128, 128), np.float16)
    m = np.arange(128)
    for q in range(T):
        out[q, 127 - m, m] = svec[q * 128 + m].astype(np.float16)
    return np.ascontiguousarray(out)


def _cvec_tiles(cvec):
    # [128, T] with [p, q] = cvec[q*128+p]
    T = len(cvec) // 128
    return np.ascontiguousarray(cvec.reshape(T, 128).T.astype(np.float32))


def _matrices():
    V2 = list(_solve_iv_split(2048))
    V3 = list(_solve_iv_split(1024))
    sig1024 = (-1.0) ** np.arange(1024)
    sig512 = (-1.0) ** np.arange(512)
    V2 = (V2[0], V2[1], sig1024 * V2[2], sig1024 * V2[3])
    V3 = (V3[0], V3[1], sig512 * V3[2], sig512 * V3[3])

    sc = np.sqrt(2.0 / S)
    M2 = sc * _CII(512)
    M2[0, :] *= np.sqrt(0.5)
    MC = sc * _CIV(512)
    MJ = MC[::-1, :].copy()

    j16 = np.eye(128, dtype=np.float16)[::-1].copy()
    return {
        "m2": _blocked(M2),
        "mc": _blocked(MC),
        "mj": _blocked(MJ),
        "j16": j16,
        "sa2": _scaled_j_blocks(V2[1]),
        "sb2": _scaled_j_blocks(V2[3]),
        "sa3": _scaled_j_blocks(V3[1]),
        "sb3": _scaled_j_blocks(V3[3]),
        "ca2": _cvec_tiles(V2[0]),
        "cb2": _cvec_tiles(V2[2]),
        "ca3": _cvec_tiles(V3[0]),
        "cb3": _cvec_tiles(V3[2]),
    }


# ---------------------------------------------------------------- bass build
def _build():
    import concourse.bacc as bacc
    import concourse.mybir as mybir
    import concourse.tile as tile

    f16 = mybir.dt.float16
    f32 = mybir.dt.float32
    CopyFn = mybir.ActivationFunctionType.Copy
    MULT = mybir.AluOpType.mult
    ADD = mybir.AluOpType.add

    nc = bacc.Bacc("TRN2", target_bir_lowering=False, debug=False, num_devices=8)
    x_d = nc.dram_tensor("x", [S, D], f16, kind="ExternalInput").ap()
    m2_d = nc.dram_tensor("m2", [4, 128, 512], f16, kind="ExternalInput").ap()
    mc_d = nc.dram_tensor("mc", [4, 128, 512], f16, kind="ExternalInput").ap()
    mj_d = nc.dram_tensor("mj", [4, 128, 512], f16, kind="ExternalInput").ap()
    j16_d = nc.dram_tensor("j16", [128, 128], f16, kind="ExternalInput").ap()
    sa2_d = nc.dram_tensor("sa2", [8, 128, 128], f16, kind="ExternalInput").ap()
    sb2_d = nc.dram_tensor("sb2", [8, 128, 128], f16, kind="ExternalInput").ap()
    sa3_d = nc.dram_tensor("sa3", [4, 128, 128], f16, kind="ExternalInput").ap()
    sb3_d = nc.dram_tensor("sb3", [4, 128, 128], f16, kind="ExternalInput").ap()
    ca2_d = nc.dram_tensor("ca2", [128, 8], f32, kind="ExternalInput").ap()
    cb2_d = nc.dram_tensor("cb2", [128, 8], f32, kind="ExternalInput").ap()
    ca3_d = nc.dram_tensor("ca3", [128, 4], f32, kind="ExternalInput").ap()
    cb3_d = nc.dram_tensor("cb3", [128, 4], f32, kind="ExternalInput").ap()
    y_d = nc.dram_tensor("y", [S, D], f16, kind="ExternalOutput").ap()
    # [e, p, kt, d]: y row 8*(kt*128+p)+e  (kt=4, p=128, e=8)
    y_pv = y_d.rearrange("(kt p e) d -> e p kt d", kt=4, p=128, e=8)
    x_pv = x_d.rearrange("(t p) d -> p t d", p=128)  # [128, 32, 1024]

    with tile.TileContext(nc) as tc:
        with (
            tc.tile_pool(name="persist", bufs=1) as persist,
            tc.tile_pool(name="stage", bufs=1) as stage,
            tc.tile_pool(name="xin", bufs=4) as xin,
            tc.tile_pool(name="jc", bufs=8) as jcp,
            tc.tile_pool(name="lf", bufs=1) as lfp,
            tc.tile_pool(name="yst", bufs=1) as yst,
            tc.tile_pool(name="ps_rev", bufs=3, space="PSUM") as ps_rev,
            tc.tile_pool(name="ps_acc", bufs=2, space="PSUM") as ps_acc,
        ):
            # --- persistent weights
            jt = persist.tile([128, 128], f16)
            nc.gpsimd.dma_start(out=jt, in_=j16_d)
            mats = {}
            for nm, dd in (("m2", m2_d), ("mc", mc_d), ("mj", mj_d)):
                t = persist.tile([128, 4, 512], f16, tag=nm)
                nc.gpsimd.dma_start(out=t, in_=dd.rearrange("kt p d -> p kt d"))
                mats[nm] = t
            sjs = {}
            for nm, dd, T in (
                ("sa2", sa2_d, 8), ("sb2", sb2_d, 8),
                ("sa3", sa3_d, 4), ("sb3", sb3_d, 4),
            ):
                t = persist.tile([128, T, 128], f16, tag=nm)
                nc.gpsimd.dma_start(out=t, in_=dd.rearrange("q p d -> p q d"))
                sjs[nm] = t
            cvs = {}
            for nm, dd, T in (
                ("ca2", ca2_d, 8), ("cb2", cb2_d, 8),
                ("ca3", ca3_d, 4), ("cb3", cb3_d, 4),
            ):
                t = persist.tile([128, T], f32, tag=nm)
                nc.gpsimd.dma_start(out=t, in_=dd)
                cvs[nm] = t

            # copy engine round-robin (PSUM -> fp16 SBUF)
            cp_state = [0]

            def psum_copy(out, in_):
                if cp_state[0] % 7 < 6:
                    nc.scalar.activation(out, in_, CopyFn)
                else:
                    nc.vector.tensor_copy(out=out, in_=in_)
                cp_state[0] += 1

            for dh in range(2):
                dsl = slice(dh * 512, (dh + 1) * 512)

                u = stage.tile([128, 16, 512], f16, tag="u")
                v = stage.tile([128, 16, 512], f16, tag="v")

                # ---- S1: u[p] = xf[p] + rev(xb[31-p]); v[p] = xf - rev
                # batches: fronts {2g,2g+1} & {14-2g,15-2g}; backs mirror
                for g in range(4):
                    fa, fb = 2 * g, 14 - 2 * g
                    xfA = xin.tile([128, 2, 512], f16, tag="xf", name="xfA")
                    nc.sync.dma_start(out=xfA, in_=x_pv[:, fa : fa + 2, dsl])
                    xbA = xin.tile([128, 2, 512], f16, tag="xb", name="xbA")
                    nc.sync.dma_start(out=xbA, in_=x_pv[:, 30 - fa : 32 - fa, dsl])
                    xfB = xin.tile([128, 2, 512], f16, tag="xf", name="xfB")
                    nc.sync.dma_start(out=xfB, in_=x_pv[:, fb : fb + 2, dsl])
                    xbB = xin.tile([128, 2, 512], f16, tag="xb", name="xbB")
                    nc.sync.dma_start(out=xbB, in_=x_pv[:, 30 - fb : 32 - fb, dsl])
                    for xf, xb, base in ((xfA, xbA, fa), (xfB, xbB, fb)):
                        for i in range(2):
                            p = base + i
                            rev = ps_rev.tile([128, 512], f32, tag="rev", name="rev")
                            nc.tensor.matmul(rev, jt, xb[:, 1 - i, :], start=True, stop=True)
                            rc = jcp.tile([128, 512], f16, tag="rc")
                            psum_copy(rc, rev)
                            nc.vector.tensor_add(u[:, p, :], xf[:, i, :], rc)
                            nc.gpsimd.tensor_sub(v[:, p, :], xf[:, i, :], rc)

                # ---- S2a: u2[q] = u[q] + rev(u[15-q]); v2[q] = u[q] - rev
                u2 = stage.tile([128, 8, 512], f16, tag="u2")
                v2 = stage.tile([128, 8, 512], f16, tag="v2")
                # ---- S2b: a[q] = ca2*v[q] + (sa2J)v[15-q]; b~ likewise
                a = stage.tile([128, 8, 512], f16, tag="a")
                bt = stage.tile([128, 8, 512], f16, tag="bt")
                for q in (0, 7, 1, 6, 2, 5, 3, 4):
                    rev = ps_rev.tile([128, 512], f32, tag="rev", name="rev")
                    nc.tensor.matmul(rev, jt, u[:, 15 - q, :], start=True, stop=True)
                    rc = jcp.tile([128, 512], f16, tag="rc")
                    psum_copy(rc, rev)
                    nc.vector.tensor_add(u2[:, q, :], u[:, q, :], rc)
                    nc.gpsimd.tensor_sub(v2[:, q, :], u[:, q, :], rc)

                    ra = ps_rev.tile([128, 512], f32, tag="rev", name="ra")
                    nc.tensor.matmul(
                        ra, sjs["sa2"][:, q, :], v[:, 15 - q, :], start=True, stop=True
                    )
                    rac = jcp.tile([128, 512], f16, tag="rc")
                    psum_copy(rac, ra)
                    nc.vector.scalar_tensor_tensor(
                        a[:, q, :], v[:, q, :], cvs["ca2"][:, q : q + 1],
                        rac, MULT, ADD,
                    )
                    rb = ps_rev.tile([128, 512], f32, tag="rev", name="rb")
                    nc.tensor.matmul(
                        rb, sjs["sb2"][:, q, :], v[:, 15 - q, :], start=True, stop=True
                    )
                    rbc = jcp.tile([128, 512], f16, tag="rc")
                    psum_copy(rbc, rb)
                    nc.vector.scalar_tensor_tensor(
                        bt[:, q, :], v[:, q, :], cvs["cb2"][:, q : q + 1],
                        rbc, MULT, ADD,
                    )

                # ---- S3: split the four 1024-size transforms to 512 leaves
                u3 = stage.tile([128, 4, 512], f16, tag="u3")
                v3 = stage.tile([128, 4, 512], f16, tag="v3")
                lvs = {}
                for nm in ("a2", "b2", "a3", "b3", "a4", "b4"):
                    lvs[nm] = stage.tile([128, 4, 512], f16, tag=nm, name=nm)
                for t in (0, 3, 1, 2):
                    rev = ps_rev.tile([128, 512], f32, tag="rev", name="rev")
                    nc.tensor.matmul(rev, jt, u2[:, 7 - t, :], start=True, stop=True)
                    rc = jcp.tile([128, 512], f16, tag="rc")
                    psum_copy(rc, rev)
                    nc.vector.tensor_add(u3[:, t, :], u2[:, t, :], rc)
                    nc.gpsimd.tensor_sub(v3[:, t, :], u2[:, t, :], rc)

                    for src, oa, ob in ((v2, "a2", "b2"), (a, "a3", "b3"), (bt, "a4", "b4")):
                        ra = ps_rev.tile([128, 512], f32, tag="rev", name="ra")
                        nc.tensor.matmul(
                            ra, sjs["sa3"][:, t, :], src[:, 7 - t, :],
                            start=True, stop=True,
                        )
                        rac = jcp.tile([128, 512], f16, tag="rc")
                        psum_copy(rac, ra)
                        nc.vector.scalar_tensor_tensor(
                            lvs[oa][:, t, :], src[:, t, :], cvs["ca3"][:, t : t + 1],
                            rac, MULT, ADD,
                        )
                        rb = ps_rev.tile([128, 512], f32, tag="rev", name="rb")
                        nc.tensor.matmul(
                            rb, sjs["sb3"][:, t, :], src[:, 7 - t, :],
                            start=True, stop=True,
                        )
                        rbc = jcp.tile([128, 512], f16, tag="rc")
                        psum_copy(rbc, rb)
                        nc.vector.scalar_tensor_tensor(
                            lvs[ob][:, t, :], src[:, t, :], cvs["cb3"][:, t : t + 1],
                            rbc, MULT, ADD,
                        )

                # ---- leaves: 8 transforms of 512, each kt-tile = sum of 4 MMs
                def leaf(mat, src, dst):
                    for kt in range(4):
                        acc = ps_acc.tile([128, 512], f32, tag=f"acc{kt % 2}")
                        for nt in range(4):
                            nc.tensor.matmul(
                                acc,
                                mats[mat][:, kt, nt * 128 : (nt + 1) * 128],
                                src[:, nt, :],
                                start=(nt == 0),
                                stop=(nt == 3),
                            )
                        psum_copy(dst[:, kt, :], acc)

                ys = {e: yst.tile([128, 4, 512], f16, tag=f"y{e}", name=f"y{e}") for e in range(8)}
                lf = {}
                for nm in ("A2", "B2", "A3", "B3", "A4", "B4"):
                    lf[nm] = lfp.tile([128, 4, 512], f16, tag=f"L{nm}", name=f"L{nm}")

                leaf("m2", u3, ys[0])
                leaf("mc", v3, ys[4])
                leaf("mc", lvs["a2"], lf["A2"])
                leaf("mj", lvs["b2"], lf["B2"])
                leaf("mc", lvs["a3"], lf["A3"])
                leaf("mj", lvs["b3"], lf["B3"])
                leaf("mj", lvs["a4"], lf["A4"])
                leaf("mc", lvs["b4"], lf["B4"])

                # ---- assembly (all fp16 SBUF)
                for kt in range(4):
                    t1 = jcp.tile([128, 512], f16, tag="tb", name="t1")
                    t2 = jcp.tile([128, 512], f16, tag="tb", name="t2")
                    t3 = jcp.tile([128, 512], f16, tag="tb", name="t3")
                    t4 = jcp.tile([128, 512], f16, tag="tb", name="t4")
                    nc.vector.tensor_add(ys[2][:, kt, :], lf["A2"][:, kt, :], lf["B2"][:, kt, :])
                    nc.gpsimd.tensor_sub(ys[6][:, kt, :], lf["A2"][:, kt, :], lf["B2"][:, kt, :])
                    nc.vector.tensor_add(t1, lf["A3"][:, kt, :], lf["B3"][:, kt, :])
                    nc.gpsimd.tensor_sub(t2, lf["A3"][:, kt, :], lf["B3"][:, kt, :])
                    nc.gpsimd.tensor_sub(t3, lf["A4"][:, kt, :], lf["B4"][:, kt, :])
                    nc.vector.tensor_add(t4, lf["A4"][:, kt, :], lf["B4"][:, kt, :])
                    nc.vector.tensor_add(ys[1][:, kt, :], t1, t3)
                    nc.gpsimd.tensor_sub(ys[3][:, kt, :], t1, t3)
                    nc.gpsimd.tensor_add(ys[5][:, kt, :], t2, t4)
                    nc.gpsimd.tensor_sub(ys[7][:, kt, :], t2, t4)

                for e in range(8):
                    nc.sync.dma_start(out=y_pv[e, :, :, dsl], in_=ys[e])

    nc.compile()
    return nc


def _get_nc():
    if "nc" not in _cache:
        _cache["nc"] = _build()
        _cache["mats"] = _matrices()
    return _cache["nc"]


def _run(x: np.ndarray, trace: bool = False):
    from concourse.bass_utils import run_bass_kernel_spmd

    nc = _get_nc()
    w = _cache["mats"]
    x16 = np.ascontiguousarray(np.asarray(x).astype(np.float16))
    in_maps = [dict(w, x=np.ascontiguousarray(x16[c])) for c in range(B)]
    res = run_bass_kernel_spmd(
        nc, in_maps, list(range(B)), trace=trace, trace_cores=[0] if trace else None
    )
    out = np.stack([res.results[c]["y"] for c in range(B)], axis=0).astype(np.float32)
    return out, res


def kernel(x: np.ndarray) -> np.ndarray:
    out, _ = _run(x, trace=False)
    return out


# revision 11
# speedup vs baseline: 1.1515x; 1.1515x over previous
"""DCT-II (norm='ortho') along axis 1 of x[8, 4096, 1024] on 8 NeuronCores.

Batch-parallel: core c transforms batch c. Depth-4 factorization:
  II_4096 -(butterfly)-> II_2048 + IV_2048
  II_2048 -(butterfly)-> II_1024 + IV_1024
  IV_2048 -(rotation V2)-> IV_1024(a) + IV_1024(b~)   [DST sign-folds in V2]
  II_1024 -(butterfly)-> II_512(u3) + IV_512(v3)
  IV_1024 -(rotation V3)-> IV_512 + IV_512            [x3 in {v2, a, b~}]
Leaves use 3 distinct 512x512 bf16 matrices (CII, CIV, J@CIV with ortho
scaling folded in); outputs assemble with lane-aligned +/- combos into
stride-8 row groups of y. Row reversals / scaled reversals run on the
tensor engine as (scaled) anti-identity matmuls; butterflies and
rotations run as bf16 tensor_tensor / scalar_tensor_tensor ops on
Vector/Scalar/GpSimd, batched 2 seq-tiles per op where possible.
x and y travel as bf16 (host converts).
"""

import sys

sys.path.insert(0, "/opt/trn_rl_repo")
import numpy as np
import ml_dtypes

B, S, D = 8, 4096, 1024
BF16 = ml_dtypes.bfloat16

_cache: dict = {}


# ---------------------------------------------------------------- host math
def _CII(n):
    k = np.arange(n)[:, None]
    m = np.arange(n)[None, :]
    return np.cos(np.pi * (2 * m + 1) * k / (2 * n))


def _CIV(n):
    k = np.arange(n)[:, None]
    m = np.arange(n)[None, :]
    return np.cos(np.pi * (2 * m + 1) * (2 * k + 1) / (4 * n))


def _DSTIV(n):
    k = np.arange(n)[:, None]
    m = np.arange(n)[None, :]
    return np.sin(np.pi * (2 * m + 1) * (2 * k + 1) / (4 * n))


def _solve_iv_split(n):
    m = n // 2
    C, Sm = _CIV(m), _DSTIV(m)
    Minv = np.zeros((n, n))
    Minv[:m, :m] = (2.0 / m) * C.T
    Minv[m:, m:] = (2.0 / m) * Sm.T
    P = np.zeros((n, n))
    i = np.arange(m)
    P[2 * i, i] = 1
    P[2 * i, m + i] = 1
    P[2 * i + 1, i] = 1
    P[2 * i + 1, m + i] = -1
    R = Minv @ (P.T / 2) @ _CIV(n)
    return (R[i, i].copy(), R[i, n - 1 - i].copy(),
            R[m + i, i].copy(), R[m + i, n - 1 - i].copy())


def _blocked(c):
    # [kt][i, nt*128+j] = C[kt*128+j, nt*128+i]  (lhsT blocks per output tile)
    kt = c.shape[0] // 128
    nt = c.shape[1] // 128
    ct = c.T.astype(BF16).reshape(nt, 128, kt, 128).transpose(2, 1, 0, 3)
    return np.ascontiguousarray(ct.reshape(kt, 128, nt * 128))


def _scaled_j_blocks(svec):
    # per 128-tile q: lhsT[k, m] = svec[q*128+m] if k == 127-m else 0
    T = len(svec) // 128
    out = np.zeros((T, 128, 128), BF16)
    m = np.arange(128)
    for q in range(T):
        out[q, 127 - m, m] = svec[q * 128 + m].astype(BF16)
    return np.ascontiguousarray(out)


def _cvec_tiles(cvec):
    # [128, T] with [p, q] = cvec[q*128+p]
    T = len(cvec) // 128
    return np.ascontiguousarray(cvec.reshape(T, 128).T.astype(np.float32))


def _matrices():
    V2 = list(_solve_iv_split(2048))
    V3 = list(_solve_iv_split(1024))
    sig1024 = (-1.0) ** np.arange(1024)
    sig512 = (-1.0) ** np.arange(512)
    V2 = (V2[0], V2[1], sig1024 * V2[2], sig1024 * V2[3])
    V3 = (V3[0], V3[1], sig512 * V3[2], sig512 * V3[3])

    sc = np.sqrt(2.0 / S)
    M2 = sc * _CII(512)
    M2[0, :] *= np.sqrt(0.5)
    MC = sc * _CIV(512)
    MJ = MC[::-1, :].copy()

    j16 = np.eye(128)[::-1].astype(BF16).copy()
    return {
        "m2": _blocked(M2),
        "mc": _blocked(MC),
        "mj": _blocked(MJ),
        "j16": j16,
        "sa2": _scaled_j_blocks(V2[1]),
        "sb2": _scaled_j_blocks(V2[3]),
        "sa3": _scaled_j_blocks(V3[1]),
        "sb3": _scaled_j_blocks(V3[3]),
        "ca2": _cvec_tiles(V2[0]),
        "cb2": _cvec_tiles(V2[2]),
        "ca3": _cvec_tiles(V3[0]),
        "cb3": _cvec_tiles(V3[2]),
    }


# ---------------------------------------------------------------- bass build
def _build():
    import concourse.bacc as bacc
    import concourse.mybir as mybir
    import concourse.tile as tile

    b16 = mybir.dt.bfloat16
    f32 = mybir.dt.float32
    CopyFn = mybir.ActivationFunctionType.Copy
    MULT = mybir.AluOpType.mult
    ADD = mybir.AluOpType.add

    nc = bacc.Bacc("TRN2", target_bir_lowering=False, debug=False, num_devices=8)
    x_d = nc.dram_tensor("x", [S, D], b16, kind="ExternalInput").ap()
    m2_d = nc.dram_tensor("m2", [4, 128, 512], b16, kind="ExternalInput").ap()
    mc_d = nc.dram_tensor("mc", [4, 128, 512], b16, kind="ExternalInput").ap()
    mj_d = nc.dram_tensor("mj", [4, 128, 512], b16, kind="ExternalInput").ap()
    j16_d = nc.dram_tensor("j16", [128, 128], b16, kind="ExternalInput").ap()
    sa2_d = nc.dram_tensor("sa2", [8, 128, 128], b16, kind="ExternalInput").ap()
    sb2_d = nc.dram_tensor("sb2", [8, 128, 128], b16, kind="ExternalInput").ap()
    sa3_d = nc.dram_tensor("sa3", [4, 128, 128], b16, kind="ExternalInput").ap()
    sb3_d = nc.dram_tensor("sb3", [4, 128, 128], b16, kind="ExternalInput").ap()
    ca2_d = nc.dram_tensor("ca2", [128, 8], f32, kind="ExternalInput").ap()
    cb2_d = nc.dram_tensor("cb2", [128, 8], f32, kind="ExternalInput").ap()
    ca3_d = nc.dram_tensor("ca3", [128, 4], f32, kind="ExternalInput").ap()
    cb3_d = nc.dram_tensor("cb3", [128, 4], f32, kind="ExternalInput").ap()
    y_d = nc.dram_tensor("y", [S, D], b16, kind="ExternalOutput").ap()
    # [e, p, kt, d]: y row 8*(kt*128+p)+e  (kt=4, p=128, e=8)
    y_pv = y_d.rearrange("(kt p e) d -> e p kt d", kt=4, p=128, e=8)
    x_pv = x_d.rearrange("(t p) d -> p t d", p=128)  # [128, 32, 1024]

    with tile.TileContext(nc) as tc:
        with (
            tc.tile_pool(name="persist", bufs=1) as persist,
            tc.tile_pool(name="stage", bufs=1) as stage,
            tc.tile_pool(name="xin", bufs=4) as xin,
            tc.tile_pool(name="jc", bufs=4) as jcp,
            tc.tile_pool(name="lf", bufs=1) as lfp,
            tc.tile_pool(name="yst", bufs=1) as yst,
            tc.tile_pool(name="ps_rev", bufs=2, space="PSUM") as ps_rev,
            tc.tile_pool(name="ps_acc", bufs=2, space="PSUM") as ps_acc,
        ):
            # --- persistent weights
            jt = persist.tile([128, 128], b16)
            nc.gpsimd.dma_start(out=jt, in_=j16_d)
            mats = {}
            for nm, dd in (("m2", m2_d), ("mc", mc_d), ("mj", mj_d)):
                t = persist.tile([128, 4, 512], b16, tag=nm, name=nm)
                nc.gpsimd.dma_start(out=t, in_=dd.rearrange("kt p d -> p kt d"))
                mats[nm] = t
            sjs = {}
            for nm, dd, T in (
                ("sa2", sa2_d, 8), ("sb2", sb2_d, 8),
                ("sa3", sa3_d, 4), ("sb3", sb3_d, 4),
            ):
                t = persist.tile([128, T, 128], b16, tag=nm, name=nm)
                nc.gpsimd.dma_start(out=t, in_=dd.rearrange("q p d -> p q d"))
                sjs[nm] = t
            cvs = {}
            for nm, dd, T in (
                ("ca2", ca2_d, 8), ("cb2", cb2_d, 8),
                ("ca3", ca3_d, 4), ("cb3", cb3_d, 4),
            ):
                t = persist.tile([128, T], f32, tag=nm, name=nm)
                nc.gpsimd.dma_start(out=t, in_=dd)
                cvs[nm] = t

            # copy engine round-robin (PSUM -> bf16 SBUF), [128, 1024] units
            cp_state = [0]

            def psum_copy(out, in_):
                if cp_state[0] % 2 == 0:
                    nc.scalar.activation(out, in_, CopyFn)
                else:
                    nc.vector.tensor_copy(out=out, in_=in_)
                cp_state[0] += 1

            for dh in range(2):
                dsl = slice(dh * 512, (dh + 1) * 512)

                u = stage.tile([128, 16, 512], b16, tag="u", name="u")
                v = stage.tile([128, 16, 512], b16, tag="v", name="v")

                # ---- S1 (2 pairs per iteration): u[p] = xf[p] + rev(xb[31-p])
                # iteration (g, half): fronts {f, f+1}, backs {31-f, 30-f}
                for f in (0, 14, 6, 8, 2, 12, 4, 10):
                    xf = xin.tile([128, 2, 512], b16, tag="xf", name="xf")
                    nc.sync.dma_start(out=xf, in_=x_pv[:, f : f + 2, dsl])
                    xb = xin.tile([128, 2, 512], b16, tag="xb", name="xb")
                    nc.sync.dma_start(out=xb, in_=x_pv[:, 30 - f : 32 - f, dsl])
                    rev = ps_rev.tile([128, 1024], f32, tag="rev", name="rev")
                    # rev[:, i*512:] = J @ xb[31-f-i] = J @ xb_tile[1-i]
                    nc.tensor.matmul(rev[:, 0:512], jt, xb[:, 1, :], start=True, stop=True)
                    nc.tensor.matmul(rev[:, 512:1024], jt, xb[:, 0, :], start=True, stop=True)
                    rc = jcp.tile([128, 1024], b16, tag="rc", name="rc")
                    psum_copy(rc, rev)
                    nc.vector.tensor_add(u[:, f : f + 2, :], xf, rc)
                    nc.gpsimd.tensor_sub(v[:, f : f + 2, :], xf, rc)

                # ---- S2a + S2b (2 pairs per iteration)
                u2 = stage.tile([128, 8, 512], b16, tag="u2", name="u2")
                v2 = stage.tile([128, 8, 512], b16, tag="v2", name="v2")
                a = stage.tile([128, 8, 512], b16, tag="a", name="a")
                bt = stage.tile([128, 8, 512], b16, tag="bt", name="bt")
                for q in (0, 6, 2, 4):
                    rev = ps_rev.tile([128, 1024], f32, tag="rev", name="rev")
                    nc.tensor.matmul(rev[:, 0:512], jt, u[:, 15 - q, :], start=True, stop=True)
                    nc.tensor.matmul(rev[:, 512:1024], jt, u[:, 14 - q, :], start=True, stop=True)
                    rc = jcp.tile([128, 1024], b16, tag="rc", name="rc")
                    psum_copy(rc, rev)
                    nc.vector.tensor_add(u2[:, q : q + 2, :], u[:, q : q + 2, :], rc)
                    nc.gpsimd.tensor_sub(v2[:, q : q + 2, :], u[:, q : q + 2, :], rc)

                    ra = ps_rev.tile([128, 1024], f32, tag="rev", name="ra")
                    nc.tensor.matmul(ra[:, 0:512], sjs["sa2"][:, q, :], v[:, 15 - q, :], start=True, stop=True)
                    nc.tensor.matmul(ra[:, 512:1024], sjs["sa2"][:, q + 1, :], v[:, 14 - q, :], start=True, stop=True)
                    rac = jcp.tile([128, 1024], b16, tag="rc", name="rac")
                    psum_copy(rac, ra)
                    rb = ps_rev.tile([128, 1024], f32, tag="rev", name="rb")
                    nc.tensor.matmul(rb[:, 0:512], sjs["sb2"][:, q, :], v[:, 15 - q, :], start=True, stop=True)
                    nc.tensor.matmul(rb[:, 512:1024], sjs["sb2"][:, q + 1, :], v[:, 14 - q, :], start=True, stop=True)
                    rbc = jcp.tile([128, 1024], b16, tag="rc", name="rbc")
                    psum_copy(rbc, rb)
                    for i in (0, 1):
                        nc.vector.scalar_tensor_tensor(
                            a[:, q + i, :], v[:, q + i, :], cvs["ca2"][:, q + i : q + i + 1],
                            rac[:, i * 512 : (i + 1) * 512], MULT, ADD)
                        nc.vector.scalar_tensor_tensor(
                            bt[:, q + i, :], v[:, q + i, :], cvs["cb2"][:, q + i : q + i + 1],
                            rbc[:, i * 512 : (i + 1) * 512], MULT, ADD)

                # ---- S3 (one 2-pair iteration per transform, t in {0,1},{2,3})
                u3 = stage.tile([128, 4, 512], b16, tag="u3", name="u3")
                v3 = stage.tile([128, 4, 512], b16, tag="v3", name="v3")
                lvs = {}
                for nm in ("a2", "b2", "a3", "b3", "a4", "b4"):
                    lvs[nm] = stage.tile([128, 4, 512], b16, tag=nm, name=nm)
                for t in (0, 2):
                    rev = ps_rev.tile([128, 1024], f32, tag="rev", name="rev")
                    nc.tensor.matmul(rev[:, 0:512], jt, u2[:, 7 - t, :], start=True, stop=True)
                    nc.tensor.matmul(rev[:, 512:1024], jt, u2[:, 6 - t, :], start=True, stop=True)
                    rc = jcp.tile([128, 1024], b16, tag="rc", name="rc")
                    psum_copy(rc, rev)
                    nc.vector.tensor_add(u3[:, t : t + 2, :], u2[:, t : t + 2, :], rc)
                    nc.gpsimd.tensor_sub(v3[:, t : t + 2, :], u2[:, t : t + 2, :], rc)

                    for src, oa, ob in ((v2, "a2", "b2"), (a, "a3", "b3"), (bt, "a4", "b4")):
                        ra = ps_rev.tile([128, 1024], f32, tag="rev", name="ra")
                        nc.tensor.matmul(ra[:, 0:512], sjs["sa3"][:, t, :], src[:, 7 - t, :], start=True, stop=True)
                        nc.tensor.matmul(ra[:, 512:1024], sjs["sa3"][:, t + 1, :], src[:, 6 - t, :], start=True, stop=True)
                        rac = jcp.tile([128, 1024], b16, tag="rc", name="rac")
                        psum_copy(rac, ra)
                        rb = ps_rev.tile([128, 1024], f32, tag="rev", name="rb")
                        nc.tensor.matmul(rb[:, 0:512], sjs["sb3"][:, t, :], src[:, 7 - t, :], start=True, stop=True)
                        nc.tensor.matmul(rb[:, 512:1024], sjs["sb3"][:, t + 1, :], src[:, 6 - t, :], start=True, stop=True)
                        rbc = jcp.tile([128, 1024], b16, tag="rc", name="rbc")
                        psum_copy(rbc, rb)
                        for i in (0, 1):
                            nc.vector.scalar_tensor_tensor(
                                lvs[oa][:, t + i, :], src[:, t + i, :], cvs["ca3"][:, t + i : t + i + 1],
                                rac[:, i * 512 : (i + 1) * 512], MULT, ADD)
                            nc.vector.scalar_tensor_tensor(
                                lvs[ob][:, t + i, :], src[:, t + i, :], cvs["cb3"][:, t + i : t + i + 1],
                                rbc[:, i * 512 : (i + 1) * 512], MULT, ADD)

                # ---- leaves: kt-pairs accumulate into [128,1024] psum
                def leaf(mat, src, dst):
                    for kth in range(2):
                        acc = ps_acc.tile([128, 1024], f32, tag="acc", name="acc")
                        for j in range(2):
                            kt = 2 * kth + j
                            for nt in range(4):
                                nc.tensor.matmul(
                                    acc[:, j * 512 : (j + 1) * 512],
                                    mats[mat][:, kt, nt * 128 : (nt + 1) * 128],
                                    src[:, nt, :],
                                    start=(nt == 0),
                                    stop=(nt == 3),
                                )
                        psum_copy(dst[:, 2 * kth : 2 * kth + 2, :], acc)

                ys = {e: yst.tile([128, 4, 512], b16, tag=f"y{e}", name=f"y{e}") for e in range(8)}
                lf = {}
                for nm in ("A2", "B2", "A3", "B3", "A4", "B4"):
                    lf[nm] = lfp.tile([128, 4, 512], b16, tag=f"L{nm}", name=f"L{nm}")

                leaf("m2", u3, ys[0])
                leaf("mc", v3, ys[4])
                leaf("mc", lvs["a2"], lf["A2"])
                leaf("mj", lvs["b2"], lf["B2"])
                leaf("mc", lvs["a3"], lf["A3"])
                leaf("mj", lvs["b3"], lf["B3"])
                leaf("mj", lvs["a4"], lf["A4"])
                leaf("mc", lvs["b4"], lf["B4"])

                # ---- assembly: one [128, 2048] op per row-group
                t1 = stage.tile([128, 4, 512], b16, tag="u3", name="t1")
                t2 = stage.tile([128, 4, 512], b16, tag="v3", name="t2")
                t3 = stage.tile([128, 4, 512], b16, tag="a2", name="t3")
                t4 = stage.tile([128, 4, 512], b16, tag="b2", name="t4")
                nc.vector.tensor_add(ys[2], lf["A2"], lf["B2"])
                nc.gpsimd.tensor_sub(ys[6], lf["A2"], lf["B2"])
                nc.vector.tensor_add(t1, lf["A3"], lf["B3"])
                nc.gpsimd.tensor_sub(t2, lf["A3"], lf["B3"])
                nc.gpsimd.tensor_sub(t3, lf["A4"], lf["B4"])
                nc.vector.tensor_add(t4, lf["A4"], lf["B4"])
                nc.vector.tensor_add(ys[1], t1, t3)
                nc.gpsimd.tensor_sub(ys[3], t1, t3)
                nc.vector.tensor_add(ys[5], t2, t4)
                nc.gpsimd.tensor_sub(ys[7], t2, t4)

                for e in range(8):
                    nc.sync.dma_start(out=y_pv[e, :, :, dsl], in_=ys[e])

    nc.compile()
    return nc


def _get_nc():
    if "nc" not in _cache:
        _cache["nc"] = _build()
        _cache["mats"] = _matrices()
    return _cache["nc"]


def _run(x: np.ndarray, trace: bool = False):
    from concourse.bass_utils import run_bass_kernel_spmd

    nc = _get_nc()
    w = _cache["mats"]
    x16 = np.ascontiguousarray(np.asarray(x, dtype=np.float32).astype(BF16))
    in_maps = [dict(w, x=np.ascontiguousarray(x16[c])) for c in range(B)]
    res = run_bass_kernel_spmd(
        nc, in_maps, list(range(B)), trace=trace, trace_cores=[0] if trace else None
    )
    out = np.stack([res.results[c]["y"] for c in range(B)], axis=0).astype(np.float32)
    return out, res


def kernel(x: np.ndarray) -> np.ndarray:
    out, _ = _run(x, trace=False)
    return out


# revision 12
# speedup vs baseline: 1.3844x; 1.2022x over previous
"""DCT-II (norm='ortho') along axis 1 of x[8, 4096, 1024] on 8 NeuronCores.

Batch-parallel: core c transforms batch c. Depth-4 factorization:
  II_4096 -(butterfly)-> II_2048 + IV_2048
  II_2048 -(butterfly)-> II_1024 + IV_1024
  IV_2048 -(rotation V2)-> IV_1024(a) + IV_1024(b~)   [DST sign-folds in V2]
  II_1024 -(butterfly)-> II_512(u3) + IV_512(v3)
  IV_1024 -(rotation V3)-> IV_512 + IV_512            [x3 in {v2, a, b~}]
Leaves use 3 distinct 512x512 bf16 matrices (CII, CIV, J@CIV with ortho
scaling folded in); outputs assemble with lane-aligned +/- combos into
stride-8 row groups of y. Row reversals / scaled reversals run on the
tensor engine as (scaled) anti-identity matmuls; butterflies and
rotations run as bf16 tensor_tensor / scalar_tensor_tensor ops on
Vector/Scalar/GpSimd, batched 2 seq-tiles per op where possible.
x and y travel as bf16 (host converts).
"""

import sys

sys.path.insert(0, "/opt/trn_rl_repo")
import numpy as np
import ml_dtypes

B, S, D = 8, 4096, 1024
BF16 = ml_dtypes.bfloat16

_cache: dict = {}


# ---------------------------------------------------------------- host math
def _CII(n):
    k = np.arange(n)[:, None]
    m = np.arange(n)[None, :]
    return np.cos(np.pi * (2 * m + 1) * k / (2 * n))


def _CIV(n):
    k = np.arange(n)[:, None]
    m = np.arange(n)[None, :]
    return np.cos(np.pi * (2 * m + 1) * (2 * k + 1) / (4 * n))


def _DSTIV(n):
    k = np.arange(n)[:, None]
    m = np.arange(n)[None, :]
    return np.sin(np.pi * (2 * m + 1) * (2 * k + 1) / (4 * n))


def _solve_iv_split(n):
    m = n // 2
    C, Sm = _CIV(m), _DSTIV(m)
    Minv = np.zeros((n, n))
    Minv[:m, :m] = (2.0 / m) * C.T
    Minv[m:, m:] = (2.0 / m) * Sm.T
    P = np.zeros((n, n))
    i = np.arange(m)
    P[2 * i, i] = 1
    P[2 * i, m + i] = 1
    P[2 * i + 1, i] = 1
    P[2 * i + 1, m + i] = -1
    R = Minv @ (P.T / 2) @ _CIV(n)
    return (R[i, i].copy(), R[i, n - 1 - i].copy(),
            R[m + i, i].copy(), R[m + i, n - 1 - i].copy())


def _blocked(c):
    # [kt][i, nt*128+j] = C[kt*128+j, nt*128+i]  (lhsT blocks per output tile)
    kt = c.shape[0] // 128
    nt = c.shape[1] // 128
    ct = c.T.astype(BF16).reshape(nt, 128, kt, 128).transpose(2, 1, 0, 3)
    return np.ascontiguousarray(ct.reshape(kt, 128, nt * 128))


def _scaled_j_blocks(svec):
    # per 128-tile q: lhsT[k, m] = svec[q*128+m] if k == 127-m else 0
    T = len(svec) // 128
    out = np.zeros((T, 128, 128), BF16)
    m = np.arange(128)
    for q in range(T):
        out[q, 127 - m, m] = svec[q * 128 + m].astype(BF16)
    return np.ascontiguousarray(out)


def _cvec_tiles(cvec):
    # [128, T] with [p, q] = cvec[q*128+p]
    T = len(cvec) // 128
    return np.ascontiguousarray(cvec.reshape(T, 128).T.astype(np.float32))


def _matrices():
    V2 = list(_solve_iv_split(2048))
    V3 = list(_solve_iv_split(1024))
    sig1024 = (-1.0) ** np.arange(1024)
    sig512 = (-1.0) ** np.arange(512)
    V2 = (V2[0], V2[1], sig1024 * V2[2], sig1024 * V2[3])
    V3 = (V3[0], V3[1], sig512 * V3[2], sig512 * V3[3])

    sc = np.sqrt(2.0 / S)
    M2 = sc * _CII(512)
    M2[0, :] *= np.sqrt(0.5)
    MC = sc * _CIV(512)
    MJ = MC[::-1, :].copy()

    j16 = np.eye(128)[::-1].astype(BF16).copy()
    return {
        "m2": _blocked(M2),
        "mc": _blocked(MC),
        "mj": _blocked(MJ),
        "j16": j16,
        "sa2": _scaled_j_blocks(V2[1]),
        "sb2": _scaled_j_blocks(V2[3]),
        "sa3": _scaled_j_blocks(V3[1]),
        "sb3": _scaled_j_blocks(V3[3]),
        "ca2": _cvec_tiles(V2[0]),
        "cb2": _cvec_tiles(V2[2]),
        "ca3": _cvec_tiles(V3[0]),
        "cb3": _cvec_tiles(V3[2]),
    }


# ---------------------------------------------------------------- bass build
def _build():
    import concourse.bacc as bacc
    import concourse.mybir as mybir
    import concourse.tile as tile

    b16 = mybir.dt.bfloat16
    f32 = mybir.dt.float32
    CopyFn = mybir.ActivationFunctionType.Copy
    MULT = mybir.AluOpType.mult
    ADD = mybir.AluOpType.add

    nc = bacc.Bacc("TRN2", target_bir_lowering=False, debug=False, num_devices=8)
    x_d = nc.dram_tensor("x", [S, D], b16, kind="ExternalInput").ap()
    m2_d = nc.dram_tensor("m2", [4, 128, 512], b16, kind="ExternalInput").ap()
    mc_d = nc.dram_tensor("mc", [4, 128, 512], b16, kind="ExternalInput").ap()
    mj_d = nc.dram_tensor("mj", [4, 128, 512], b16, kind="ExternalInput").ap()
    j16_d = nc.dram_tensor("j16", [128, 128], b16, kind="ExternalInput").ap()
    sa2_d = nc.dram_tensor("sa2", [8, 128, 128], b16, kind="ExternalInput").ap()
    sb2_d = nc.dram_tensor("sb2", [8, 128, 128], b16, kind="ExternalInput").ap()
    sa3_d = nc.dram_tensor("sa3", [4, 128, 128], b16, kind="ExternalInput").ap()
    sb3_d = nc.dram_tensor("sb3", [4, 128, 128], b16, kind="ExternalInput").ap()
    ca2_d = nc.dram_tensor("ca2", [128, 8], f32, kind="ExternalInput").ap()
    cb2_d = nc.dram_tensor("cb2", [128, 8], f32, kind="ExternalInput").ap()
    ca3_d = nc.dram_tensor("ca3", [128, 4], f32, kind="ExternalInput").ap()
    cb3_d = nc.dram_tensor("cb3", [128, 4], f32, kind="ExternalInput").ap()
    y_d = nc.dram_tensor("y", [S, D], b16, kind="ExternalOutput").ap()
    # [e, p, kt, d]: y row 8*(kt*128+p)+e  (kt=4, p=128, e=8)
    y_pv = y_d.rearrange("(kt p e) d -> e p kt d", kt=4, p=128, e=8)
    x_pv = x_d.rearrange("(t p) d -> p t d", p=128)  # [128, 32, 1024]

    with tile.TileContext(nc) as tc:
        with (
            tc.tile_pool(name="persist", bufs=1) as persist,
            tc.tile_pool(name="stage", bufs=1) as stage,
            tc.tile_pool(name="xin", bufs=4) as xin,
            tc.tile_pool(name="jc", bufs=4) as jcp,
            tc.tile_pool(name="lf", bufs=1) as lfp,
            tc.tile_pool(name="yst", bufs=1) as yst,
            tc.tile_pool(name="ps_rev", bufs=2, space="PSUM") as ps_rev,
            tc.tile_pool(name="ps_acc", bufs=2, space="PSUM") as ps_acc,
        ):
            # --- persistent weights
            jt = persist.tile([128, 128], b16)
            nc.gpsimd.dma_start(out=jt, in_=j16_d)
            mats = {}
            for nm, dd in (("m2", m2_d), ("mc", mc_d), ("mj", mj_d)):
                t = persist.tile([128, 4, 512], b16, tag=nm, name=nm)
                nc.gpsimd.dma_start(out=t, in_=dd.rearrange("kt p d -> p kt d"))
                mats[nm] = t
            sjs = {}
            for nm, dd, T in (
                ("sa2", sa2_d, 8), ("sb2", sb2_d, 8),
                ("sa3", sa3_d, 4), ("sb3", sb3_d, 4),
            ):
                t = persist.tile([128, T, 128], b16, tag=nm, name=nm)
                nc.gpsimd.dma_start(out=t, in_=dd.rearrange("q p d -> p q d"))
                sjs[nm] = t
            cvs = {}
            for nm, dd, T in (
                ("ca2", ca2_d, 8), ("cb2", cb2_d, 8),
                ("ca3", ca3_d, 4), ("cb3", cb3_d, 4),
            ):
                t = persist.tile([128, T], f32, tag=nm, name=nm)
                nc.gpsimd.dma_start(out=t, in_=dd)
                cvs[nm] = t

            # copy engine round-robin (PSUM -> bf16 SBUF), [128, 1024] units
            cp_state = [0]

            def psum_copy(out, in_):
                if cp_state[0] % 2 == 0:
                    nc.scalar.activation(out, in_, CopyFn)
                else:
                    nc.vector.tensor_copy(out=out, in_=in_)
                cp_state[0] += 1

            for dh in range(2):
                dsl = slice(dh * 512, (dh + 1) * 512)

                u = stage.tile([128, 16, 512], b16, tag="u", name="u")
                v = stage.tile([128, 16, 512], b16, tag="v", name="v")

                # ---- S1 (2 pairs per iteration): u[p] = xf[p] + rev(xb[31-p])
                # iteration (g, half): fronts {f, f+1}, backs {31-f, 30-f}
                for f in (0, 14, 6, 8, 2, 12, 4, 10):
                    xf = xin.tile([128, 2, 512], b16, tag="xf", name="xf")
                    nc.sync.dma_start(out=xf, in_=x_pv[:, f : f + 2, dsl])
                    xb = xin.tile([128, 2, 512], b16, tag="xb", name="xb")
                    nc.sync.dma_start(out=xb, in_=x_pv[:, 30 - f : 32 - f, dsl])
                    rev = ps_rev.tile([128, 1024], f32, tag="rev", name="rev")
                    # rev[:, i*512:] = J @ xb[31-f-i] = J @ xb_tile[1-i]
                    nc.tensor.matmul(rev[:, 0:512], jt, xb[:, 1, :], start=True, stop=True)
                    nc.tensor.matmul(rev[:, 512:1024], jt, xb[:, 0, :], start=True, stop=True)
                    rc = jcp.tile([128, 1024], b16, tag="rc", name="rc")
                    psum_copy(rc, rev)
                    nc.vector.tensor_add(u[:, f : f + 2, :], xf, rc)
                    nc.vector.tensor_sub(v[:, f : f + 2, :], xf, rc)

                # ---- S2a + S2b (2 pairs per iteration)
                u2 = stage.tile([128, 8, 512], b16, tag="u2", name="u2")
                v2 = stage.tile([128, 8, 512], b16, tag="v2", name="v2")
                a = stage.tile([128, 8, 512], b16, tag="a", name="a")
                bt = stage.tile([128, 8, 512], b16, tag="bt", name="bt")
                for q in (0, 6, 2, 4):
                    rev = ps_rev.tile([128, 1024], f32, tag="rev", name="rev")
                    nc.tensor.matmul(rev[:, 0:512], jt, u[:, 15 - q, :], start=True, stop=True)
                    nc.tensor.matmul(rev[:, 512:1024], jt, u[:, 14 - q, :], start=True, stop=True)
                    rc = jcp.tile([128, 1024], b16, tag="rc", name="rc")
                    psum_copy(rc, rev)
                    nc.vector.tensor_add(u2[:, q : q + 2, :], u[:, q : q + 2, :], rc)
                    nc.vector.tensor_sub(v2[:, q : q + 2, :], u[:, q : q + 2, :], rc)

                    ra = ps_rev.tile([128, 1024], f32, tag="rev", name="ra")
                    nc.tensor.matmul(ra[:, 0:512], sjs["sa2"][:, q, :], v[:, 15 - q, :], start=True, stop=True)
                    nc.tensor.matmul(ra[:, 512:1024], sjs["sa2"][:, q + 1, :], v[:, 14 - q, :], start=True, stop=True)
                    rb = ps_rev.tile([128, 1024], f32, tag="rev", name="rb")
                    nc.tensor.matmul(rb[:, 0:512], sjs["sb2"][:, q, :], v[:, 15 - q, :], start=True, stop=True)
                    nc.tensor.matmul(rb[:, 512:1024], sjs["sb2"][:, q + 1, :], v[:, 14 - q, :], start=True, stop=True)
                    for i in (0, 1):
                        nc.vector.scalar_tensor_tensor(
                            a[:, q + i, :], v[:, q + i, :], cvs["ca2"][:, q + i : q + i + 1],
                            ra[:, i * 512 : (i + 1) * 512], MULT, ADD)
                        nc.vector.scalar_tensor_tensor(
                            bt[:, q + i, :], v[:, q + i, :], cvs["cb2"][:, q + i : q + i + 1],
                            rb[:, i * 512 : (i + 1) * 512], MULT, ADD)

                # ---- S3 (one 2-pair iteration per transform, t in {0,1},{2,3})
                u3 = stage.tile([128, 4, 512], b16, tag="u3", name="u3")
                v3 = stage.tile([128, 4, 512], b16, tag="v3", name="v3")
                lvs = {}
                for nm in ("a2", "b2", "a3", "b3", "a4", "b4"):
                    lvs[nm] = stage.tile([128, 4, 512], b16, tag=nm, name=nm)
                for t in (0, 2):
                    rev = ps_rev.tile([128, 1024], f32, tag="rev", name="rev")
                    nc.tensor.matmul(rev[:, 0:512], jt, u2[:, 7 - t, :], start=True, stop=True)
                    nc.tensor.matmul(rev[:, 512:1024], jt, u2[:, 6 - t, :], start=True, stop=True)
                    rc = jcp.tile([128, 1024], b16, tag="rc", name="rc")
                    psum_copy(rc, rev)
                    nc.vector.tensor_add(u3[:, t : t + 2, :], u2[:, t : t + 2, :], rc)
                    nc.vector.tensor_sub(v3[:, t : t + 2, :], u2[:, t : t + 2, :], rc)

                    for src, oa, ob in ((v2, "a2", "b2"), (a, "a3", "b3"), (bt, "a4", "b4")):
                        ra = ps_rev.tile([128, 1024], f32, tag="rev", name="ra")
                        nc.tensor.matmul(ra[:, 0:512], sjs["sa3"][:, t, :], src[:, 7 - t, :], start=True, stop=True)
                        nc.tensor.matmul(ra[:, 512:1024], sjs["sa3"][:, t + 1, :], src[:, 6 - t, :], start=True, stop=True)
                        rb = ps_rev.tile([128, 1024], f32, tag="rev", name="rb")
                        nc.tensor.matmul(rb[:, 0:512], sjs["sb3"][:, t, :], src[:, 7 - t, :], start=True, stop=True)
                        nc.tensor.matmul(rb[:, 512:1024], sjs["sb3"][:, t + 1, :], src[:, 6 - t, :], start=True, stop=True)
                        for i in (0, 1):
                            nc.vector.scalar_tensor_tensor(
                                lvs[oa][:, t + i, :], src[:, t + i, :], cvs["ca3"][:, t + i : t + i + 1],
                                ra[:, i * 512 : (i + 1) * 512], MULT, ADD)
                            nc.vector.scalar_tensor_tensor(
                                lvs[ob][:, t + i, :], src[:, t + i, :], cvs["cb3"][:, t + i : t + i + 1],
                                rb[:, i * 512 : (i + 1) * 512], MULT, ADD)

                # ---- leaves: kt-pairs accumulate into [128,1024] psum
                def leaf(mat, src, dst):
                    for kth in range(2):
                        acc = ps_acc.tile([128, 1024], f32, tag="acc", name="acc")
                        for j in range(2):
                            kt = 2 * kth + j
                            for nt in range(4):
                                nc.tensor.matmul(
                                    acc[:, j * 512 : (j + 1) * 512],
                                    mats[mat][:, kt, nt * 128 : (nt + 1) * 128],
                                    src[:, nt, :],
                                    start=(nt == 0),
                                    stop=(nt == 3),
                                )
                        psum_copy(dst[:, 2 * kth : 2 * kth + 2, :], acc)

                ys = {e: yst.tile([128, 4, 512], b16, tag=f"y{e}", name=f"y{e}") for e in range(8)}
                lf = {}
                for nm in ("A2", "B2", "A3", "B3", "A4", "B4"):
                    lf[nm] = lfp.tile([128, 4, 512], b16, tag=f"L{nm}", name=f"L{nm}")

                leaf("m2", u3, ys[0])
                leaf("mc", v3, ys[4])
                leaf("mc", lvs["a2"], lf["A2"])
                leaf("mj", lvs["b2"], lf["B2"])
                leaf("mc", lvs["a3"], lf["A3"])
                leaf("mj", lvs["b3"], lf["B3"])
                leaf("mj", lvs["a4"], lf["A4"])
                leaf("mc", lvs["b4"], lf["B4"])

                # ---- assembly: one [128, 2048] op per row-group
                t1 = stage.tile([128, 4, 512], b16, tag="u3", name="t1")
                t2 = stage.tile([128, 4, 512], b16, tag="v3", name="t2")
                t3 = stage.tile([128, 4, 512], b16, tag="a2", name="t3")
                t4 = stage.tile([128, 4, 512], b16, tag="b2", name="t4")
                nc.vector.tensor_add(ys[2], lf["A2"], lf["B2"])
                nc.vector.tensor_sub(ys[6], lf["A2"], lf["B2"])
                nc.vector.tensor_add(t1, lf["A3"], lf["B3"])
                nc.vector.tensor_sub(t2, lf["A3"], lf["B3"])
                nc.vector.tensor_sub(t3, lf["A4"], lf["B4"])
                nc.vector.tensor_add(t4, lf["A4"], lf["B4"])
                nc.vector.tensor_add(ys[1], t1, t3)
                nc.vector.tensor_sub(ys[3], t1, t3)
                nc.vector.tensor_add(ys[5], t2, t4)
                nc.vector.tensor_sub(ys[7], t2, t4)

                for e in range(8):
                    nc.gpsimd.dma_start(out=y_pv[e, :, :, dsl], in_=ys[e])

    nc.compile()
    return nc


def _get_nc():
    if "nc" not in _cache:
        _cache["nc"] = _build()
        _cache["mats"] = _matrices()
    return _cache["nc"]


def _run(x: np.ndarray, trace: bool = False):
    from concourse.bass_utils import run_bass_kernel_spmd

    nc = _get_nc()
    w = _cache["mats"]
    x16 = np.ascontiguousarray(np.asarray(x, dtype=np.float32).astype(BF16))
    in_maps = [dict(w, x=np.ascontiguousarray(x16[c])) for c in range(B)]
    res = run_bass_kernel_spmd(
        nc, in_maps, list(range(B)), trace=trace, trace_cores=[0] if trace else None
    )
    out = np.stack([res.results[c]["y"] for c in range(B)], axis=0).astype(np.float32)
    return out, res


def kernel(x: np.ndarray) -> np.ndarray:
    out, _ = _run(x, trace=False)
    return out
